# revision 46
# baseline (speedup 1.0000x reference)
"""Trainium2 Bass kernel for nn_BlockDecomposition (relational GNN message passing).

Reference computation:
    out[n] = keep[n] * (x[n] @ BD(blocks[-1]))                    (self loop)
           + sum_{directed edge e: tgt_e == n} w_e * (x[src_e] @ BD(blocks[et_e]))
where BD(.) embeds 32 4x4 blocks into a block-diagonal 128x128 matrix and the
edge list is symmetrized (each undirected edge appears in both directions).

Strategy (8 NeuronCores, no collectives):
  - Shard by TARGET node: core c owns nodes [c*1250, (c+1)*1250). Each core
    receives exactly the directed edges targeting its nodes (plus one
    self-loop pseudo-edge per node with relation 16 and weight keep[n]),
    computes its 1250 output rows completely, and the host concatenates.
  - Within a core, nodes are processed in 10 blocks of 128. Per block one
    dma_gather (GPSIMD SWDGE) pulls all needed x rows from the HBM-resident
    fp16 x table into SBUF, laid out [edge mod 128 (partition), tile, 128
    features] -- the gather IS the edge-expansion of x.
  - Relations are organized per block into supergroups of <=4 relation
    "slots" sharing a [din, 4*128] PSUM bank. Each relation contributes
    floor(gmax/128) dense 128-edge "full" tiles; the <=127-edge remainders
    of a supergroup are concatenated into shared 512-wide "merged" tiles
    (one-hot column = 128*slot + tgt_local), eliminating per-relation tail
    padding. Per tile:
      * DVE builds a weighted one-hot OH[e, col] = (iota[col] ==
        tloc[e]) * w[e] in ONE fused tensor_scalar (is_equal, mult), fp16.
      * PE scatter-matmul aggT[din, col] += xg[e, din].T-contract OH[e, col]
        (fp16 x fp16, fp32 PSUM accumulate; 1 cycle/row).
    Per supergroup: one ACT copy moves the PSUM bank to SBUF as fp16; then
    per relation a PE transform matmul out[n, dout] += agg[n, din] @
    BD(W_r)[din, dout] accumulates all 17 relations in a per-block PSUM
    bank, which is copied out (ACT) and DMA'd to the output rows.
  - The schedule (tile counts per cell) is the max over the 8 cores so a
    single SPMD program serves all cores; shorter cores pad with weight-0
    edges. Self-loops ride the same path as relation 16 with w = keep mask.
  - Engine balance (cost model, per core ~53us): DVE ~39us (one-hots),
    Pool ~39us (gather descriptor-gen), PE ~38us (530 matmuls), ACT ~32us
    (PSUM->SBUF copies), all overlapped against ~6.6us/block gather DMA.

Numerics: gathered x, one-hots, and block weights are fp16 (measured HW
matmul rel-err ~3e-4; end-to-end 4.0e-4 vs fp64 reference); accumulation is
fp32 in PSUM. All floating-point arithmetic happens on device. Host work is
index manipulation (sorting/padding/layout), dtype casts, and placing weight
values into the block-diagonal layout.
"""

import os
import sys
import numpy as np

for _p in ("/opt/trn_rl_repo", "/root/.axon_site/_ro/trn_rl_repo"):
    if os.path.isdir(_p) and _p not in sys.path:
        sys.path.insert(0, _p)

import concourse.bass as bass
import concourse.bacc as bacc
import concourse.mybir as mybir
import concourse.tile as tile
from concourse.bass_utils import run_bass_kernel_spmd

# ----------------------------------------------------------------------------
# Problem constants (hardcoded per spec)
N_NODES = 10000
N_EDGES = 160000
NUM_REL = 16          # relations used by edges; blocks[16] is the self-loop
NUM_BLOCKS = 32
BLOCK_SIZE = 4
D = NUM_BLOCKS * BLOCK_SIZE   # 128
N_CORES = 8
NPC = N_NODES // N_CORES      # 1250 nodes per core
BLK = 128                     # node block size (partition dim of scatter)
NBLK = (NPC + BLK - 1) // BLK  # 10 blocks per core (last one partial: 98)
NRELS = NUM_REL + 1           # 16 edge relations + self-loop "relation" 16
TILE_E = 128                  # edges per tile (matmul contraction dim)

F32 = mybir.dt.float32
F16 = mybir.dt.float16
I16 = mybir.dt.int16

# fraction of one-hot builds routed to the GPSIMD (Pool) engine to unload DVE
POOL_OH_EVERY = 1000  # Pool does DMA desc-gen only; all one-hots on DVE

_DEBUG_SIM = os.environ.get("KERNEL_USE_CORESIM", "0") == "1"


# ----------------------------------------------------------------------------
# Host-side preprocessing: integer index manipulation only.

SUPERGROUPS = [list(range(4 * g, 4 * g + 4)) for g in range(4)] + [[NUM_REL]]


def _build_schedule(cnt):
    """Static tile schedule shared by all cores.

    cnt: [C, NBLK, NRELS] per-core (block, rel) edge counts.

    Per block, relations are organized into supergroups of <=4 relation
    "slots" sharing one [din, 512] PSUM bank (slot j at columns 128j). Each
    relation cell contributes floor(gmax/128) dense "full" tiles targeting
    its slot plus a remainder; remainders of a supergroup are concatenated
    and chopped into shared 512-wide "merged" tiles (each edge's one-hot
    column is 128*slot + tloc), which removes per-relation tail padding.

    Returns (sched, Ttot):
      sched: per block dict {
        "sgs": [ { "rels": [r...], "slots": {r: j},
                   "tiles": [ (kind, width, start, stop) ... ]   # in order
                   "cells": {r: (full_tiles, rem)} } ] }
      Ttot: total tile count.
    """
    gmax = cnt.max(axis=0)  # [NBLK, NRELS]
    sched = []
    Ttot = 0
    for b in range(NBLK):
        sgs = []
        for rels_all in SUPERGROUPS:
            rels = [r for r in rels_all if gmax[b, r] > 0]
            if not rels:
                continue
            slots = {r: j for j, r in enumerate(rels)}
            full = {r: int(gmax[b, r]) // TILE_E for r in rels}
            rem = {r: int(gmax[b, r]) % TILE_E for r in rels}
            rem_total = sum(rem.values())
            m = (rem_total + TILE_E - 1) // TILE_E
            n_full = sum(full.values())
            tiles = []
            for i in range(m):
                tiles.append(("merged", TILE_E, i == 0, False))
            for r in rels:
                for t in range(full[r]):
                    # with merged tiles the first merged matmul resets the
                    # whole bank (start), and group bookkeeping is skipped;
                    # without, each slot runs its own start/stop group
                    tiles.append(
                        (
                            "full_%d" % slots[r],
                            TILE_E,
                            m == 0 and t == 0,
                            m == 0 and t == full[r] - 1,
                        )
                    )
            if m > 0:
                tiles[-1] = (tiles[-1][0], tiles[-1][1], tiles[-1][2], True)
            sgs.append(
                {
                    "rels": rels,
                    "slots": slots,
                    "full": full,
                    "rem": rem,
                    "m": m,
                    "ntiles": len(tiles),
                    "tiles": tiles,
                }
            )
            Ttot += len(tiles)
        sched.append({"sgs": sgs})
    return sched, Ttot


def _preprocess(x, node_keep_mask, source, target, edge_type, edge_weights):
    """Build the per-core padded tile schedule.

    Returns:
      sched, Ttot (see _build_schedule), plus per-core arrays:
        src_pad  [C, Ttot*128] int16   source node id per edge slot
        tloc_pad [C, Ttot*128] float32 one-hot column per edge slot
                                        (0..127 full tiles, 0..511 merged)
        w_pad    [C, Ttot*128] float32 edge weight per edge slot (0 for pads)
    """
    src = np.asarray(source).astype(np.int64)
    tgt = np.asarray(target).astype(np.int64)
    et = np.asarray(edge_type).astype(np.int64)
    ew = np.asarray(edge_weights).astype(np.float32)
    keep = np.asarray(node_keep_mask).astype(np.float32)

    # symmetrize + append self-loop pseudo-edges with relation NUM_REL
    nodes = np.arange(N_NODES, dtype=np.int64)
    srcA = np.concatenate([src, tgt, nodes])
    tgtA = np.concatenate([tgt, src, nodes])
    etA = np.concatenate([et, et, np.full(N_NODES, NUM_REL, dtype=np.int64)])
    ewA = np.concatenate([ew, ew, keep])

    core = tgtA // NPC
    loc = tgtA % NPC
    blk = loc // BLK
    tloc = loc % BLK

    # sort by (core, blk, rel); order within a group is irrelevant
    order = np.lexsort((etA, blk, core))
    srcS = srcA[order].astype(np.int16)
    tlocS = tloc[order].astype(np.float32)
    ewS = ewA[order]

    key = (core * NBLK + blk) * NRELS + etA
    cnt = np.bincount(key, minlength=N_CORES * NBLK * NRELS).reshape(
        N_CORES, NBLK, NRELS
    )
    starts = np.concatenate([[0], np.cumsum(cnt.reshape(-1))]).astype(np.int64)

    sched, Ttot = _build_schedule(cnt)

    src_pad = np.zeros((N_CORES, Ttot * TILE_E), dtype=np.int16)
    tloc_pad = np.zeros((N_CORES, Ttot * TILE_E), dtype=np.float32)
    w_pad = np.zeros((N_CORES, Ttot * TILE_E), dtype=np.float32)

    for c in range(N_CORES):
        pos = 0  # edge-slot cursor within this core's stream
        for b in range(NBLK):
            for sg in sched[b]["sgs"]:
                # per-rel edge lists for this core
                seg = {}
                for r in sg["rels"]:
                    gi = (c * NBLK + b) * NRELS + r
                    s0 = int(starts[gi])
                    n = int(cnt[c, b, r])
                    seg[r] = (s0, n)
                # fill order: merged region first (remainder slots of each
                # rel = the edges beyond the full tiles), then full tiles.
                mslots = sg["m"] * TILE_E
                fbase = pos + mslots  # full-tile region start
                # merged region layout: concat over rels of rem[r] slots
                moff = pos
                for r in sg["rels"]:
                    s0, n = seg[r]
                    j = sg["slots"][r]
                    nfull_slots = sg["full"][r] * TILE_E
                    # full tiles take the first min(n, nfull_slots) edges
                    nf = min(n, nfull_slots)
                    src_pad[c, fbase : fbase + nf] = srcS[s0 : s0 + nf]
                    tloc_pad[c, fbase : fbase + nf] = tlocS[s0 : s0 + nf]
                    w_pad[c, fbase : fbase + nf] = ewS[s0 : s0 + nf]
                    fbase += nfull_slots
                    # remainder edges go to this rel's merged slots with
                    # one-hot column 128*j + tloc
                    nr = n - nf
                    assert 0 <= nr <= sg["rem"][r]
                    src_pad[c, moff : moff + nr] = srcS[s0 + nf : s0 + n]
                    tloc_pad[c, moff : moff + nr] = (
                        tlocS[s0 + nf : s0 + n] + 128.0 * j
                    )
                    w_pad[c, moff : moff + nr] = ewS[s0 + nf : s0 + n]
                    moff += sg["rem"][r]
                pos += sg["ntiles"] * TILE_E
        assert pos == Ttot * TILE_E
    return sched, Ttot, src_pad, tloc_pad, w_pad


def _make_bdw(blocks):
    """blocks [17, 32, 4, 4] -> dense block-diagonal lhsT layout [128, 17*128]
    with BDW[:, r*128:(r+1)*128][4b+i, 4b+j] = blocks[r, b, i, j]."""
    blocks = np.asarray(blocks).astype(np.float32)
    bdw = np.zeros((D, NRELS * D), dtype=np.float32)
    for r in range(NRELS):
        for b in range(NUM_BLOCKS):
            bdw[
                b * BLOCK_SIZE : (b + 1) * BLOCK_SIZE,
                r * D + b * BLOCK_SIZE : r * D + (b + 1) * BLOCK_SIZE,
            ] = blocks[r, b]
    return bdw


def _tiles_per_block(sched):
    return [sum(sg["ntiles"] for sg in blk["sgs"]) for blk in sched]


def _wrap_idxs(src_pad_core, tiles_per_block):
    """Pack per-block gather indices in the dma_gather wrapped layout:
    index j of a block lives at [j % 16, j // 16], replicated across the 8
    groups of 16 partitions. Blocks are concatenated along the free dim.
    Returns [128, Ttot*8] int16."""
    cols = []
    off = 0
    for tb in tiles_per_block:
        ni = int(tb) * TILE_E
        seg = src_pad_core[off : off + ni]
        wrapped = seg.reshape(ni // 16, 16).T  # [16, ni//16]
        cols.append(np.tile(wrapped, (8, 1)))  # [128, ni//16]
        off += ni
    return np.ascontiguousarray(np.concatenate(cols, axis=1))


# ----------------------------------------------------------------------------
# Bass kernel builder (one SPMD program for all cores)

def _build_nc(sched, Ttot):
    tiles_per_block = _tiles_per_block(sched)

    # Bacc (not raw Bass): its compile() pass splits multi-sem waits into
    # EventSemaphores (TRN2 allows 1 wait/instruction), auto-inserts GPSIMD
    # library loads for dma_gather, and encodes extended InstISA subclasses.
    nc = bacc.Bacc("TRN2", target_bir_lowering=False, debug=False, num_devices=N_CORES)

    # fp16 datapath: x table, one-hots, and block-diag weights are fp16
    # (measured matmul rel-err ~3e-4); PSUM accumulation stays fp32.
    # fp16 matmuls run at 1 cycle/row vs 4 for fp32.
    x_d = nc.declare_dram_parameter("x16", [N_NODES, D], F16, isOutput=False)
    srcidx_d = nc.declare_dram_parameter("srcidx", [128, Ttot * 8], I16, isOutput=False)
    # metaf packs [tloc | w] (fp32 tensor_scalar operands) into one DMA;
    # meta16 packs [iota512 | bdw] (fp16). Consumers then depend on few DMAs
    # (ISA sync-wait slots per instruction are scarce).
    metaf_cols = 2 * Ttot
    metaf_d = nc.declare_dram_parameter("metaf", [128, metaf_cols], F32, isOutput=False)
    meta16_cols = 512 + NRELS * D
    meta16_d = nc.declare_dram_parameter("meta16", [128, meta16_cols], F16, isOutput=False)
    out_d = nc.declare_dram_parameter("out", [NBLK * BLK, D], F32, isOutput=True)

    with tile.TileContext(nc) as tc:
        with (
            tc.tile_pool(name="const", bufs=1) as const_pool,
            tc.tile_pool(name="xg", bufs=3) as xg_pool,
            tc.tile_pool(name="oh", bufs=6) as oh_pool,
            tc.tile_pool(name="aggsb", bufs=3) as aggsb_pool,
            tc.tile_pool(name="outsb", bufs=2) as outsb_pool,
            tc.tile_pool(name="psA", bufs=3, space=bass.MemorySpace.PSUM) as psA_pool,
            tc.tile_pool(name="psO", bufs=2, space=bass.MemorySpace.PSUM) as psO_pool,
        ):
            # constants
            srcidx_sb = const_pool.tile([128, Ttot * 8], I16, tag="srcidx")
            nc.sync.dma_start(srcidx_sb[:], srcidx_d[:, :])
            metaf_sb = const_pool.tile([128, metaf_cols], F32, tag="metaf")
            nc.sync.dma_start(metaf_sb[:], metaf_d[:, :])
            meta16_sb = const_pool.tile([128, meta16_cols], F16, tag="meta16")
            nc.sync.dma_start(meta16_sb[:], meta16_d[:, :])
            tloc_sb = metaf_sb[:, 0:Ttot]
            w_sb = metaf_sb[:, Ttot : 2 * Ttot]
            iota_sb = meta16_sb[:, 0:512]
            bdw_sb = meta16_sb[:, 512:]

            tcol = 0       # global tile counter (column into tloc/w)
            scol = 0       # column offset into srcidx (8 cols per tile)
            max_tb = max(tiles_per_block)
            xg_off = 0
            for b in range(NBLK):
                tb = tiles_per_block[b]
                if tb == 0:
                    continue
                ni = tb * TILE_E
                # gather all source rows for this block: [e%128, e//128, din]
                xg = xg_pool.tile([128, max_tb, D], F16, tag="xg")
                nc.gpsimd.dma_gather(
                    out_ap=xg[:, :tb, :],
                    in_ap=x_d[:, :],
                    idxs_ap=srcidx_sb[:, scol : scol + tb * 8],
                    num_idxs=ni,
                    num_idxs_reg=ni,
                    elem_size=D,
                    # single_packet=True caps the index payload at one 2KB
                    # packet (1024 int16 idxs); crashes the device beyond
                    single_packet=False,
                )
                scol += tb * 8
                xg_off = 0

                out_ps = psO_pool.tile([BLK, D], F32, tag="outps")
                n_transforms = sum(len(sg["rels"]) for sg in sched[b]["sgs"])
                gt = xg_off   # tile index within the pair gather
                ti = 0        # transform index within block
                for sg in sched[b]["sgs"]:
                    nslots = len(sg["rels"])
                    mixed = sg["m"] > 0  # merged tiles present
                    agg_ps = psA_pool.tile([D, 4 * BLK], F32, tag="aggps")
                    for kind, width, start, stop in sg["tiles"]:
                        if kind == "merged":
                            oh_w = 4 * BLK
                            tgt_ap = agg_ps[:]
                        else:
                            j = int(kind.split("_")[1])
                            oh_w = BLK
                            tgt_ap = agg_ps[:, j * BLK : (j + 1) * BLK]
                        oh = oh_pool.tile([128, 4 * BLK], F16, tag="oh")
                        oh_eng = (
                            nc.gpsimd
                            if (tcol % POOL_OH_EVERY == POOL_OH_EVERY - 1)
                            else nc.vector
                        )
                        oh_eng.tensor_scalar(
                            oh[:, :oh_w],
                            iota_sb[:, :oh_w],
                            tloc_sb[:, tcol : tcol + 1],
                            w_sb[:, tcol : tcol + 1],
                            mybir.AluOpType.is_equal,
                            mybir.AluOpType.mult,
                        )
                        # aggT[din, col] += sum_e xg[e, din] * oh[e, col]
                        nc.tensor.matmul(
                            tgt_ap,
                            xg[:, gt, :],
                            oh[:, :oh_w],
                            start=start,
                            stop=stop,
                            skip_group_check=mixed,
                        )
                        tcol += 1
                        gt += 1
                    used = nslots * BLK
                    agg_sb = aggsb_pool.tile([D, 4 * BLK], F16, tag="aggsb")
                    nc.scalar.copy(agg_sb[:, :used], agg_ps[:, :used])
                    for r in sg["rels"]:
                        j = sg["slots"][r]
                        # out[n, dout] += agg[n, din] @ BDW_r[din, dout]
                        nc.tensor.matmul(
                            out_ps[:],
                            agg_sb[:, j * BLK : (j + 1) * BLK],
                            bdw_sb[:, r * D : (r + 1) * D],
                            start=(ti == 0),
                            stop=(ti == n_transforms - 1),
                        )
                        ti += 1
                xg_off = gt
                out_sb = outsb_pool.tile([BLK, D], F32, tag="outsb")
                nc.scalar.copy(out_sb[:], out_ps[:])
                nc.sync.dma_start(out_d[b * BLK : (b + 1) * BLK, :], out_sb[:])
    nc.compile()
    return nc


# ----------------------------------------------------------------------------

def _make_in_maps(x, sched, Ttot, src_pad, tloc_pad, w_pad, blocks):
    bdw = _make_bdw(blocks)
    iota512 = np.tile(np.arange(512, dtype=np.float32)[None, :], (128, 1))
    tpb = _tiles_per_block(sched)

    x16 = x.astype(np.float16)
    meta16 = np.ascontiguousarray(
        np.concatenate([iota512, bdw], axis=1).astype(np.float16)
    )
    in_maps = []
    for c in range(N_CORES):
        metaf = np.concatenate(
            [tloc_pad[c].reshape(Ttot, 128).T, w_pad[c].reshape(Ttot, 128).T],
            axis=1,
        )
        in_maps.append(
            {
                "x16": x16,
                "srcidx": _wrap_idxs(src_pad[c], tpb),
                "metaf": np.ascontiguousarray(metaf),
                "meta16": meta16,
            }
        )
    return in_maps


def kernel(x, node_keep_mask, source, target, edge_type, edge_weights, blocks):
    global LAST_NC, LAST_IN_MAPS
    x = np.ascontiguousarray(np.asarray(x), dtype=np.float32)
    sched, Ttot, src_pad, tloc_pad, w_pad = _preprocess(
        x, node_keep_mask, source, target, edge_type, edge_weights
    )
    in_maps = _make_in_maps(x, sched, Ttot, src_pad, tloc_pad, w_pad, blocks)
    nc = _build_nc(sched, Ttot)
    LAST_NC, LAST_IN_MAPS = nc, in_maps

    if _DEBUG_SIM:
        from concourse.bass_interp import CoreSim

        outs = []
        for c in range(N_CORES):
            sim = CoreSim(nc)
            for k, v in in_maps[c].items():
                sim.tensor(k)[:] = v
            sim.simulate()
            outs.append(np.array(sim.tensor("out"))[:NPC])
        return np.concatenate(outs, axis=0)

    trace = os.environ.get("KERNEL_TRACE", "0") == "1"
    res = run_bass_kernel_spmd(
        nc, in_maps, core_ids=list(range(N_CORES)), trace=trace
    )
    global LAST_EXEC_TIME_NS
    LAST_EXEC_TIME_NS = res.exec_time_ns
    out = np.concatenate([res.results[c]["out"][:NPC] for c in range(N_CORES)], axis=0)
    return out.astype(np.float32)


LAST_EXEC_TIME_NS = None
LAST_NC = None
LAST_IN_MAPS = None


# revision 48
# speedup vs baseline: 1.0150x; 1.0150x over previous
"""Trainium2 Bass kernel for nn_BlockDecomposition (relational GNN message passing).

Reference computation:
    out[n] = keep[n] * (x[n] @ BD(blocks[-1]))                    (self loop)
           + sum_{directed edge e: tgt_e == n} w_e * (x[src_e] @ BD(blocks[et_e]))
where BD(.) embeds 32 4x4 blocks into a block-diagonal 128x128 matrix and the
edge list is symmetrized (each undirected edge appears in both directions).

Strategy (8 NeuronCores, no collectives):
  - Shard by TARGET node: core c owns nodes [c*1250, (c+1)*1250). Each core
    receives exactly the directed edges targeting its nodes (plus one
    self-loop pseudo-edge per node with relation 16 and weight keep[n]),
    computes its 1250 output rows completely, and the host concatenates.
  - Within a core, nodes are processed in 10 blocks of 128. Per block one
    dma_gather (GPSIMD SWDGE) pulls all needed x rows from the HBM-resident
    fp16 x table into SBUF, laid out [edge mod 128 (partition), tile, 128
    features] -- the gather IS the edge-expansion of x.
  - Relations are organized per block into supergroups of <=4 relation
    "slots" sharing a [din, 4*128] PSUM bank. Each relation contributes
    floor(gmax/128) dense 128-edge "full" tiles; the <=127-edge remainders
    of a supergroup are concatenated into shared 512-wide "merged" tiles
    (one-hot column = 128*slot + tgt_local), eliminating per-relation tail
    padding. Per tile:
      * DVE builds a weighted one-hot OH[e, col] = (iota[col] ==
        tloc[e]) * w[e] in ONE fused tensor_scalar (is_equal, mult), fp16.
      * PE scatter-matmul aggT[din, col] += xg[e, din].T-contract OH[e, col]
        (fp16 x fp16, fp32 PSUM accumulate; 1 cycle/row).
    Per supergroup: one ACT copy moves the PSUM bank to SBUF as fp16; then
    per relation a PE transform matmul out[n, dout] += agg[n, din] @
    BD(W_r)[din, dout] accumulates all 17 relations in a per-block PSUM
    bank, which is copied out (ACT) and DMA'd to the output rows.
  - The schedule (tile counts per cell) is the max over the 8 cores so a
    single SPMD program serves all cores; shorter cores pad with weight-0
    edges. Self-loops ride the same path as relation 16 with w = keep mask.
  - Engine balance (cost model, per core ~53us): DVE ~39us (one-hots),
    Pool ~39us (gather descriptor-gen), PE ~38us (530 matmuls), ACT ~32us
    (PSUM->SBUF copies), all overlapped against ~6.6us/block gather DMA.

Numerics: gathered x, one-hots, and block weights are fp16 (measured HW
matmul rel-err ~3e-4; end-to-end 4.0e-4 vs fp64 reference); accumulation is
fp32 in PSUM. All floating-point arithmetic happens on device. Host work is
index manipulation (sorting/padding/layout), dtype casts, and placing weight
values into the block-diagonal layout.
"""

import os
import sys
import numpy as np

for _p in ("/opt/trn_rl_repo", "/root/.axon_site/_ro/trn_rl_repo"):
    if os.path.isdir(_p) and _p not in sys.path:
        sys.path.insert(0, _p)

import concourse.bass as bass
import concourse.bacc as bacc
import concourse.mybir as mybir
import concourse.tile as tile
from concourse.bass_utils import run_bass_kernel_spmd

# ----------------------------------------------------------------------------
# Problem constants (hardcoded per spec)
N_NODES = 10000
N_EDGES = 160000
NUM_REL = 16          # relations used by edges; blocks[16] is the self-loop
NUM_BLOCKS = 32
BLOCK_SIZE = 4
D = NUM_BLOCKS * BLOCK_SIZE   # 128
N_CORES = 8
NPC = N_NODES // N_CORES      # 1250 nodes per core
BLK = 128                     # node block size (partition dim of scatter)
NBLK = (NPC + BLK - 1) // BLK  # 10 blocks per core (last one partial: 98)
NRELS = NUM_REL + 1           # 16 edge relations + self-loop "relation" 16
TILE_E = 128                  # edges per tile (matmul contraction dim)

F32 = mybir.dt.float32
F16 = mybir.dt.float16
I16 = mybir.dt.int16

# fraction of one-hot builds routed to the GPSIMD (Pool) engine to unload DVE
POOL_OH_EVERY = 1000  # Pool does DMA desc-gen only; all one-hots on DVE

_DEBUG_SIM = os.environ.get("KERNEL_USE_CORESIM", "0") == "1"


# ----------------------------------------------------------------------------
# Host-side preprocessing: integer index manipulation only.

SUPERGROUPS = [list(range(4 * g, 4 * g + 4)) for g in range(4)] + [[NUM_REL]]


def _build_schedule(cnt):
    """Static tile schedule shared by all cores.

    cnt: [C, NBLK, NRELS] per-core (block, rel) edge counts.

    Per block, relations are organized into supergroups of <=4 relation
    "slots" sharing one [din, 512] PSUM bank (slot j at columns 128j). Each
    relation cell contributes floor(gmax/128) dense "full" tiles targeting
    its slot plus a remainder; remainders of a supergroup are concatenated
    and chopped into shared 512-wide "merged" tiles (each edge's one-hot
    column is 128*slot + tloc), which removes per-relation tail padding.

    Returns (sched, Ttot):
      sched: per block dict {
        "sgs": [ { "rels": [r...], "slots": {r: j},
                   "tiles": [ (kind, width, start, stop) ... ]   # in order
                   "cells": {r: (full_tiles, rem)} } ] }
      Ttot: total tile count.
    """
    gmax = cnt.max(axis=0)  # [NBLK, NRELS]
    sched = []
    Ttot = 0
    for b in range(NBLK):
        sgs = []
        for rels_all in SUPERGROUPS:
            rels = [r for r in rels_all if gmax[b, r] > 0]
            if not rels:
                continue
            slots = {r: j for j, r in enumerate(rels)}
            full = {r: int(gmax[b, r]) // TILE_E for r in rels}
            rem = {r: int(gmax[b, r]) % TILE_E for r in rels}
            rem_total = sum(rem.values())
            m = (rem_total + TILE_E - 1) // TILE_E
            n_full = sum(full.values())
            tiles = []
            for i in range(m):
                tiles.append(("merged", TILE_E, i == 0, False))
            for r in rels:
                for t in range(full[r]):
                    # with merged tiles the first merged matmul resets the
                    # whole bank (start), and group bookkeeping is skipped;
                    # without, each slot runs its own start/stop group
                    tiles.append(
                        (
                            "full_%d" % slots[r],
                            TILE_E,
                            m == 0 and t == 0,
                            m == 0 and t == full[r] - 1,
                        )
                    )
            if m > 0:
                tiles[-1] = (tiles[-1][0], tiles[-1][1], tiles[-1][2], True)
            sgs.append(
                {
                    "rels": rels,
                    "slots": slots,
                    "full": full,
                    "rem": rem,
                    "m": m,
                    "ntiles": len(tiles),
                    "tiles": tiles,
                }
            )
            Ttot += len(tiles)
        sched.append({"sgs": sgs})
    return sched, Ttot


def _preprocess(x, node_keep_mask, source, target, edge_type, edge_weights):
    """Build the per-core padded tile schedule.

    Returns:
      sched, Ttot (see _build_schedule), plus per-core arrays:
        src_pad  [C, Ttot*128] int16   source node id per edge slot
        tloc_pad [C, Ttot*128] float32 one-hot column per edge slot
                                        (0..127 full tiles, 0..511 merged)
        w_pad    [C, Ttot*128] float32 edge weight per edge slot (0 for pads)
    """
    src = np.asarray(source).astype(np.int64)
    tgt = np.asarray(target).astype(np.int64)
    et = np.asarray(edge_type).astype(np.int64)
    ew = np.asarray(edge_weights).astype(np.float32)
    keep = np.asarray(node_keep_mask).astype(np.float32)

    # symmetrize + append self-loop pseudo-edges with relation NUM_REL
    nodes = np.arange(N_NODES, dtype=np.int64)
    srcA = np.concatenate([src, tgt, nodes])
    tgtA = np.concatenate([tgt, src, nodes])
    etA = np.concatenate([et, et, np.full(N_NODES, NUM_REL, dtype=np.int64)])
    ewA = np.concatenate([ew, ew, keep])

    core = tgtA // NPC
    loc = tgtA % NPC
    blk = loc // BLK
    tloc = loc % BLK

    # sort by (core, blk, rel); order within a group is irrelevant
    order = np.lexsort((etA, blk, core))
    srcS = srcA[order].astype(np.int16)
    tlocS = tloc[order].astype(np.float32)
    ewS = ewA[order]

    key = (core * NBLK + blk) * NRELS + etA
    cnt = np.bincount(key, minlength=N_CORES * NBLK * NRELS).reshape(
        N_CORES, NBLK, NRELS
    )
    starts = np.concatenate([[0], np.cumsum(cnt.reshape(-1))]).astype(np.int64)

    sched, Ttot = _build_schedule(cnt)

    src_pad = np.zeros((N_CORES, Ttot * TILE_E), dtype=np.int16)
    tloc_pad = np.zeros((N_CORES, Ttot * TILE_E), dtype=np.float32)
    w_pad = np.zeros((N_CORES, Ttot * TILE_E), dtype=np.float32)

    for c in range(N_CORES):
        pos = 0  # edge-slot cursor within this core's stream
        for b in range(NBLK):
            for sg in sched[b]["sgs"]:
                # per-rel edge lists for this core
                seg = {}
                for r in sg["rels"]:
                    gi = (c * NBLK + b) * NRELS + r
                    s0 = int(starts[gi])
                    n = int(cnt[c, b, r])
                    seg[r] = (s0, n)
                # fill order: merged region first (remainder slots of each
                # rel = the edges beyond the full tiles), then full tiles.
                mslots = sg["m"] * TILE_E
                fbase = pos + mslots  # full-tile region start
                # merged region layout: concat over rels of rem[r] slots
                moff = pos
                for r in sg["rels"]:
                    s0, n = seg[r]
                    j = sg["slots"][r]
                    nfull_slots = sg["full"][r] * TILE_E
                    # full tiles take the first min(n, nfull_slots) edges
                    nf = min(n, nfull_slots)
                    src_pad[c, fbase : fbase + nf] = srcS[s0 : s0 + nf]
                    tloc_pad[c, fbase : fbase + nf] = tlocS[s0 : s0 + nf]
                    w_pad[c, fbase : fbase + nf] = ewS[s0 : s0 + nf]
                    fbase += nfull_slots
                    # remainder edges go to this rel's merged slots with
                    # one-hot column 128*j + tloc
                    nr = n - nf
                    assert 0 <= nr <= sg["rem"][r]
                    src_pad[c, moff : moff + nr] = srcS[s0 + nf : s0 + n]
                    tloc_pad[c, moff : moff + nr] = (
                        tlocS[s0 + nf : s0 + n] + 128.0 * j
                    )
                    w_pad[c, moff : moff + nr] = ewS[s0 + nf : s0 + n]
                    moff += sg["rem"][r]
                pos += sg["ntiles"] * TILE_E
        assert pos == Ttot * TILE_E
    return sched, Ttot, src_pad, tloc_pad, w_pad


def _make_bdw(blocks):
    """blocks [17, 32, 4, 4] -> dense block-diagonal lhsT layout [128, 17*128]
    with BDW[:, r*128:(r+1)*128][4b+i, 4b+j] = blocks[r, b, i, j]."""
    blocks = np.asarray(blocks).astype(np.float32)
    bdw = np.zeros((D, NRELS * D), dtype=np.float32)
    for r in range(NRELS):
        for b in range(NUM_BLOCKS):
            bdw[
                b * BLOCK_SIZE : (b + 1) * BLOCK_SIZE,
                r * D + b * BLOCK_SIZE : r * D + (b + 1) * BLOCK_SIZE,
            ] = blocks[r, b]
    return bdw


def _tiles_per_block(sched):
    return [sum(sg["ntiles"] for sg in blk["sgs"]) for blk in sched]


def _wrap_idxs(src_pad_core, tiles_per_block):
    """Pack per-block gather indices in the dma_gather wrapped layout:
    index j of a block lives at [j % 16, j // 16], replicated across the 8
    groups of 16 partitions. Blocks are concatenated along the free dim.
    Returns [128, Ttot*8] int16."""
    cols = []
    off = 0
    for tb in tiles_per_block:
        ni = int(tb) * TILE_E
        seg = src_pad_core[off : off + ni]
        wrapped = seg.reshape(ni // 16, 16).T  # [16, ni//16]
        cols.append(np.tile(wrapped, (8, 1)))  # [128, ni//16]
        off += ni
    return np.ascontiguousarray(np.concatenate(cols, axis=1))


# ----------------------------------------------------------------------------
# Bass kernel builder (one SPMD program for all cores)

def _build_nc(sched, Ttot):
    tiles_per_block = _tiles_per_block(sched)

    # Bacc (not raw Bass): its compile() pass splits multi-sem waits into
    # EventSemaphores (TRN2 allows 1 wait/instruction), auto-inserts GPSIMD
    # library loads for dma_gather, and encodes extended InstISA subclasses.
    nc = bacc.Bacc("TRN2", target_bir_lowering=False, debug=False, num_devices=N_CORES)

    # fp16 datapath: x table, one-hots, and block-diag weights are fp16
    # (measured matmul rel-err ~3e-4); PSUM accumulation stays fp32.
    # fp16 matmuls run at 1 cycle/row vs 4 for fp32.
    x_d = nc.declare_dram_parameter("x16", [N_NODES, D], F16, isOutput=False)
    srcidx_d = nc.declare_dram_parameter("srcidx", [128, Ttot * 8], I16, isOutput=False)
    # metaf packs [tloc | w] (fp32 tensor_scalar operands) into one DMA;
    # meta16 packs [iota512 | bdw] (fp16). Consumers then depend on few DMAs
    # (ISA sync-wait slots per instruction are scarce).
    metaf_cols = 2 * Ttot
    metaf_d = nc.declare_dram_parameter("metaf", [128, metaf_cols], F32, isOutput=False)
    meta16_cols = 512 + NRELS * D
    meta16_d = nc.declare_dram_parameter("meta16", [128, meta16_cols], F16, isOutput=False)
    out_d = nc.declare_dram_parameter("out", [NBLK * BLK, D], F32, isOutput=True)

    with tile.TileContext(nc) as tc:
        with (
            tc.tile_pool(name="const", bufs=1) as const_pool,
            tc.tile_pool(name="xg", bufs=3) as xg_pool,
            tc.tile_pool(name="oh", bufs=6) as oh_pool,
            tc.tile_pool(name="aggsb", bufs=3) as aggsb_pool,
            tc.tile_pool(name="outsb", bufs=2) as outsb_pool,
            tc.tile_pool(name="psA", bufs=6, space=bass.MemorySpace.PSUM) as psA_pool,
            tc.tile_pool(name="psO", bufs=2, space=bass.MemorySpace.PSUM) as psO_pool,
        ):
            # constants
            srcidx_sb = const_pool.tile([128, Ttot * 8], I16, tag="srcidx")
            nc.sync.dma_start(srcidx_sb[:], srcidx_d[:, :])
            metaf_sb = const_pool.tile([128, metaf_cols], F32, tag="metaf")
            nc.sync.dma_start(metaf_sb[:], metaf_d[:, :])
            meta16_sb = const_pool.tile([128, meta16_cols], F16, tag="meta16")
            nc.sync.dma_start(meta16_sb[:], meta16_d[:, :])
            tloc_sb = metaf_sb[:, 0:Ttot]
            w_sb = metaf_sb[:, Ttot : 2 * Ttot]
            iota_sb = meta16_sb[:, 0:512]
            bdw_sb = meta16_sb[:, 512:]

            tcol = 0       # global tile counter (column into tloc/w)
            scol = 0       # column offset into srcidx (8 cols per tile)
            max_tb = max(tiles_per_block)
            xg_off = 0
            for b in range(NBLK):
                tb = tiles_per_block[b]
                if tb == 0:
                    continue
                ni = tb * TILE_E
                # gather all source rows for this block: [e%128, e//128, din]
                xg = xg_pool.tile([128, max_tb, D], F16, tag="xg")
                nc.gpsimd.dma_gather(
                    out_ap=xg[:, :tb, :],
                    in_ap=x_d[:, :],
                    idxs_ap=srcidx_sb[:, scol : scol + tb * 8],
                    num_idxs=ni,
                    num_idxs_reg=ni,
                    elem_size=D,
                    # single_packet=True caps the index payload at one 2KB
                    # packet (1024 int16 idxs); crashes the device beyond
                    single_packet=False,
                )
                scol += tb * 8
                xg_off = 0

                out_ps = psO_pool.tile([BLK, D], F32, tag="outps")
                n_transforms = sum(len(sg["rels"]) for sg in sched[b]["sgs"])
                gt = xg_off   # tile index within the block gather
                ti = 0        # transform index within block
                # phase 1: all scatter matmuls of the block (keeps every
                # supergroup's PSUM bank live so PE never stalls behind an
                # ACT copy mid-block)
                pending = []
                for sg in sched[b]["sgs"]:
                    mixed = sg["m"] > 0  # merged tiles present
                    agg_ps = psA_pool.tile([D, 4 * BLK], F32, tag="aggps")
                    pending.append((sg, agg_ps))
                    for kind, width, start, stop in sg["tiles"]:
                        if kind == "merged":
                            oh_w = 4 * BLK
                            tgt_ap = agg_ps[:]
                        else:
                            j = int(kind.split("_")[1])
                            oh_w = BLK
                            tgt_ap = agg_ps[:, j * BLK : (j + 1) * BLK]
                        oh = oh_pool.tile([128, 4 * BLK], F16, tag="oh")
                        oh_eng = (
                            nc.gpsimd
                            if (tcol % POOL_OH_EVERY == POOL_OH_EVERY - 1)
                            else nc.vector
                        )
                        oh_eng.tensor_scalar(
                            oh[:, :oh_w],
                            iota_sb[:, :oh_w],
                            tloc_sb[:, tcol : tcol + 1],
                            w_sb[:, tcol : tcol + 1],
                            mybir.AluOpType.is_equal,
                            mybir.AluOpType.mult,
                        )
                        # aggT[din, col] += sum_e xg[e, din] * oh[e, col]
                        nc.tensor.matmul(
                            tgt_ap,
                            xg[:, gt, :],
                            oh[:, :oh_w],
                            start=start,
                            stop=stop,
                            skip_group_check=mixed,
                        )
                        tcol += 1
                        gt += 1
                # phase 2: PSUM->SBUF copies + transform matmuls
                for sg, agg_ps in pending:
                    used = len(sg["rels"]) * BLK
                    agg_sb = aggsb_pool.tile([D, 4 * BLK], F16, tag="aggsb")
                    nc.scalar.copy(agg_sb[:, :used], agg_ps[:, :used])
                    for r in sg["rels"]:
                        j = sg["slots"][r]
                        # out[n, dout] += agg[n, din] @ BDW_r[din, dout]
                        nc.tensor.matmul(
                            out_ps[:],
                            agg_sb[:, j * BLK : (j + 1) * BLK],
                            bdw_sb[:, r * D : (r + 1) * D],
                            start=(ti == 0),
                            stop=(ti == n_transforms - 1),
                        )
                        ti += 1
                xg_off = gt
                out_sb = outsb_pool.tile([BLK, D], F32, tag="outsb")
                nc.scalar.copy(out_sb[:], out_ps[:])
                nc.sync.dma_start(out_d[b * BLK : (b + 1) * BLK, :], out_sb[:])
    nc.compile()
    return nc


# ----------------------------------------------------------------------------

def _make_in_maps(x, sched, Ttot, src_pad, tloc_pad, w_pad, blocks):
    bdw = _make_bdw(blocks)
    iota512 = np.tile(np.arange(512, dtype=np.float32)[None, :], (128, 1))
    tpb = _tiles_per_block(sched)

    x16 = x.astype(np.float16)
    meta16 = np.ascontiguousarray(
        np.concatenate([iota512, bdw], axis=1).astype(np.float16)
    )
    in_maps = []
    for c in range(N_CORES):
        metaf = np.concatenate(
            [tloc_pad[c].reshape(Ttot, 128).T, w_pad[c].reshape(Ttot, 128).T],
            axis=1,
        )
        in_maps.append(
            {
                "x16": x16,
                "srcidx": _wrap_idxs(src_pad[c], tpb),
                "metaf": np.ascontiguousarray(metaf),
                "meta16": meta16,
            }
        )
    return in_maps


def kernel(x, node_keep_mask, source, target, edge_type, edge_weights, blocks):
    global LAST_NC, LAST_IN_MAPS
    x = np.ascontiguousarray(np.asarray(x), dtype=np.float32)
    sched, Ttot, src_pad, tloc_pad, w_pad = _preprocess(
        x, node_keep_mask, source, target, edge_type, edge_weights
    )
    in_maps = _make_in_maps(x, sched, Ttot, src_pad, tloc_pad, w_pad, blocks)
    nc = _build_nc(sched, Ttot)
    LAST_NC, LAST_IN_MAPS = nc, in_maps

    if _DEBUG_SIM:
        from concourse.bass_interp import CoreSim

        outs = []
        for c in range(N_CORES):
            sim = CoreSim(nc)
            for k, v in in_maps[c].items():
                sim.tensor(k)[:] = v
            sim.simulate()
            outs.append(np.array(sim.tensor("out"))[:NPC])
        return np.concatenate(outs, axis=0)

    trace = os.environ.get("KERNEL_TRACE", "0") == "1"
    res = run_bass_kernel_spmd(
        nc, in_maps, core_ids=list(range(N_CORES)), trace=trace
    )
    global LAST_EXEC_TIME_NS
    LAST_EXEC_TIME_NS = res.exec_time_ns
    out = np.concatenate([res.results[c]["out"][:NPC] for c in range(N_CORES)], axis=0)
    return out.astype(np.float32)


LAST_EXEC_TIME_NS = None
LAST_NC = None
LAST_IN_MAPS = None


# revision 49
# speedup vs baseline: 1.0210x; 1.0059x over previous
"""Trainium2 Bass kernel for nn_BlockDecomposition (relational GNN message passing).

Reference computation:
    out[n] = keep[n] * (x[n] @ BD(blocks[-1]))                    (self loop)
           + sum_{directed edge e: tgt_e == n} w_e * (x[src_e] @ BD(blocks[et_e]))
where BD(.) embeds 32 4x4 blocks into a block-diagonal 128x128 matrix and the
edge list is symmetrized (each undirected edge appears in both directions).

Strategy (8 NeuronCores, no collectives):
  - Shard by TARGET node: core c owns nodes [c*1250, (c+1)*1250). Each core
    receives exactly the directed edges targeting its nodes (plus one
    self-loop pseudo-edge per node with relation 16 and weight keep[n]),
    computes its 1250 output rows completely, and the host concatenates.
  - Within a core, nodes are processed in 10 blocks of 128. Per block one
    dma_gather (GPSIMD SWDGE) pulls all needed x rows from the HBM-resident
    fp16 x table into SBUF, laid out [edge mod 128 (partition), tile, 128
    features] -- the gather IS the edge-expansion of x.
  - Relations are organized per block into supergroups of <=4 relation
    "slots" sharing a [din, 4*128] PSUM bank. Each relation contributes
    floor(gmax/128) dense 128-edge "full" tiles; the <=127-edge remainders
    of a supergroup are concatenated into shared 512-wide "merged" tiles
    (one-hot column = 128*slot + tgt_local), eliminating per-relation tail
    padding. Per tile:
      * DVE builds a weighted one-hot OH[e, col] = (iota[col] ==
        tloc[e]) * w[e] in ONE fused tensor_scalar (is_equal, mult), fp16.
      * PE scatter-matmul aggT[din, col] += xg[e, din].T-contract OH[e, col]
        (fp16 x fp16, fp32 PSUM accumulate; 1 cycle/row).
    Per supergroup: one ACT copy moves the PSUM bank to SBUF as fp16; then
    per relation a PE transform matmul out[n, dout] += agg[n, din] @
    BD(W_r)[din, dout] accumulates all 17 relations in a per-block PSUM
    bank, which is copied out (ACT) and DMA'd to the output rows.
  - The schedule (tile counts per cell) is the max over the 8 cores so a
    single SPMD program serves all cores; shorter cores pad with weight-0
    edges. Self-loops ride the same path as relation 16 with w = keep mask.
  - Engine balance (cost model, per core ~53us): DVE ~39us (one-hots),
    Pool ~39us (gather descriptor-gen), PE ~38us (530 matmuls), ACT ~32us
    (PSUM->SBUF copies), all overlapped against ~6.6us/block gather DMA.

Numerics: gathered x, one-hots, and block weights are fp16 (measured HW
matmul rel-err ~3e-4; end-to-end 4.0e-4 vs fp64 reference); accumulation is
fp32 in PSUM. All floating-point arithmetic happens on device. Host work is
index manipulation (sorting/padding/layout), dtype casts, and placing weight
values into the block-diagonal layout.
"""

import os
import sys
import numpy as np

for _p in ("/opt/trn_rl_repo", "/root/.axon_site/_ro/trn_rl_repo"):
    if os.path.isdir(_p) and _p not in sys.path:
        sys.path.insert(0, _p)

import concourse.bass as bass
import concourse.bacc as bacc
import concourse.mybir as mybir
import concourse.tile as tile
from concourse.bass_utils import run_bass_kernel_spmd

# ----------------------------------------------------------------------------
# Problem constants (hardcoded per spec)
N_NODES = 10000
N_EDGES = 160000
NUM_REL = 16          # relations used by edges; blocks[16] is the self-loop
NUM_BLOCKS = 32
BLOCK_SIZE = 4
D = NUM_BLOCKS * BLOCK_SIZE   # 128
N_CORES = 8
NPC = N_NODES // N_CORES      # 1250 nodes per core
BLK = 128                     # node block size (partition dim of scatter)
NBLK = (NPC + BLK - 1) // BLK  # 10 blocks per core (last one partial: 98)
NRELS = NUM_REL + 1           # 16 edge relations + self-loop "relation" 16
TILE_E = 128                  # edges per tile (matmul contraction dim)

F32 = mybir.dt.float32
F16 = mybir.dt.float16
I16 = mybir.dt.int16

# fraction of one-hot builds routed to the GPSIMD (Pool) engine to unload DVE
POOL_OH_EVERY = 1000  # Pool does DMA desc-gen only; all one-hots on DVE

_DEBUG_SIM = os.environ.get("KERNEL_USE_CORESIM", "0") == "1"


# ----------------------------------------------------------------------------
# Host-side preprocessing: integer index manipulation only.

SUPERGROUPS = [list(range(4 * g, 4 * g + 4)) for g in range(4)] + [[NUM_REL]]


def _build_schedule(cnt):
    """Static tile schedule shared by all cores.

    cnt: [C, NBLK, NRELS] per-core (block, rel) edge counts.

    Per block, relations are organized into supergroups of <=4 relation
    "slots" sharing one [din, 512] PSUM bank (slot j at columns 128j). Each
    relation cell contributes floor(gmax/128) dense "full" tiles targeting
    its slot plus a remainder; remainders of a supergroup are concatenated
    and chopped into shared 512-wide "merged" tiles (each edge's one-hot
    column is 128*slot + tloc), which removes per-relation tail padding.

    Returns (sched, Ttot):
      sched: per block dict {
        "sgs": [ { "rels": [r...], "slots": {r: j},
                   "tiles": [ (kind, width, start, stop) ... ]   # in order
                   "cells": {r: (full_tiles, rem)} } ] }
      Ttot: total tile count.
    """
    gmax = cnt.max(axis=0)  # [NBLK, NRELS]
    sched = []
    Ttot = 0
    for b in range(NBLK):
        sgs = []
        for rels_all in SUPERGROUPS:
            rels = [r for r in rels_all if gmax[b, r] > 0]
            if not rels:
                continue
            slots = {r: j for j, r in enumerate(rels)}
            full = {r: int(gmax[b, r]) // TILE_E for r in rels}
            rem = {r: int(gmax[b, r]) % TILE_E for r in rels}
            rem_total = sum(rem.values())
            m = (rem_total + TILE_E - 1) // TILE_E
            n_full = sum(full.values())
            tiles = []
            for i in range(m):
                tiles.append(("merged", TILE_E, i == 0, False))
            for r in rels:
                for t in range(full[r]):
                    # with merged tiles the first merged matmul resets the
                    # whole bank (start), and group bookkeeping is skipped;
                    # without, each slot runs its own start/stop group
                    tiles.append(
                        (
                            "full_%d" % slots[r],
                            TILE_E,
                            m == 0 and t == 0,
                            m == 0 and t == full[r] - 1,
                        )
                    )
            if m > 0:
                tiles[-1] = (tiles[-1][0], tiles[-1][1], tiles[-1][2], True)
            sgs.append(
                {
                    "rels": rels,
                    "slots": slots,
                    "full": full,
                    "rem": rem,
                    "m": m,
                    "ntiles": len(tiles),
                    "tiles": tiles,
                }
            )
            Ttot += len(tiles)
        sched.append({"sgs": sgs})
    return sched, Ttot


def _preprocess(x, node_keep_mask, source, target, edge_type, edge_weights):
    """Build the per-core padded tile schedule.

    Returns:
      sched, Ttot (see _build_schedule), plus per-core arrays:
        src_pad  [C, Ttot*128] int16   source node id per edge slot
        tloc_pad [C, Ttot*128] float32 one-hot column per edge slot
                                        (0..127 full tiles, 0..511 merged)
        w_pad    [C, Ttot*128] float32 edge weight per edge slot (0 for pads)
    """
    src = np.asarray(source).astype(np.int64)
    tgt = np.asarray(target).astype(np.int64)
    et = np.asarray(edge_type).astype(np.int64)
    ew = np.asarray(edge_weights).astype(np.float32)
    keep = np.asarray(node_keep_mask).astype(np.float32)

    # symmetrize + append self-loop pseudo-edges with relation NUM_REL
    nodes = np.arange(N_NODES, dtype=np.int64)
    srcA = np.concatenate([src, tgt, nodes])
    tgtA = np.concatenate([tgt, src, nodes])
    etA = np.concatenate([et, et, np.full(N_NODES, NUM_REL, dtype=np.int64)])
    ewA = np.concatenate([ew, ew, keep])

    core = tgtA // NPC
    loc = tgtA % NPC
    blk = loc // BLK
    tloc = loc % BLK

    # sort by (core, blk, rel); order within a group is irrelevant
    order = np.lexsort((etA, blk, core))
    srcS = srcA[order].astype(np.int16)
    tlocS = tloc[order].astype(np.float32)
    ewS = ewA[order]

    key = (core * NBLK + blk) * NRELS + etA
    cnt = np.bincount(key, minlength=N_CORES * NBLK * NRELS).reshape(
        N_CORES, NBLK, NRELS
    )
    starts = np.concatenate([[0], np.cumsum(cnt.reshape(-1))]).astype(np.int64)

    sched, Ttot = _build_schedule(cnt)

    src_pad = np.zeros((N_CORES, Ttot * TILE_E), dtype=np.int16)
    tloc_pad = np.zeros((N_CORES, Ttot * TILE_E), dtype=np.float32)
    w_pad = np.zeros((N_CORES, Ttot * TILE_E), dtype=np.float32)

    for c in range(N_CORES):
        pos = 0  # edge-slot cursor within this core's stream
        for b in range(NBLK):
            for sg in sched[b]["sgs"]:
                # per-rel edge lists for this core
                seg = {}
                for r in sg["rels"]:
                    gi = (c * NBLK + b) * NRELS + r
                    s0 = int(starts[gi])
                    n = int(cnt[c, b, r])
                    seg[r] = (s0, n)
                # fill order: merged region first (remainder slots of each
                # rel = the edges beyond the full tiles), then full tiles.
                mslots = sg["m"] * TILE_E
                fbase = pos + mslots  # full-tile region start
                # merged region layout: concat over rels of rem[r] slots
                moff = pos
                for r in sg["rels"]:
                    s0, n = seg[r]
                    j = sg["slots"][r]
                    nfull_slots = sg["full"][r] * TILE_E
                    # full tiles take the first min(n, nfull_slots) edges
                    nf = min(n, nfull_slots)
                    src_pad[c, fbase : fbase + nf] = srcS[s0 : s0 + nf]
                    tloc_pad[c, fbase : fbase + nf] = tlocS[s0 : s0 + nf]
                    w_pad[c, fbase : fbase + nf] = ewS[s0 : s0 + nf]
                    fbase += nfull_slots
                    # remainder edges go to this rel's merged slots with
                    # one-hot column 128*j + tloc
                    nr = n - nf
                    assert 0 <= nr <= sg["rem"][r]
                    src_pad[c, moff : moff + nr] = srcS[s0 + nf : s0 + n]
                    tloc_pad[c, moff : moff + nr] = (
                        tlocS[s0 + nf : s0 + n] + 128.0 * j
                    )
                    w_pad[c, moff : moff + nr] = ewS[s0 + nf : s0 + n]
                    moff += sg["rem"][r]
                pos += sg["ntiles"] * TILE_E
        assert pos == Ttot * TILE_E
    return sched, Ttot, src_pad, tloc_pad, w_pad


def _make_bdw(blocks):
    """blocks [17, 32, 4, 4] -> dense block-diagonal lhsT layout [128, 17*128]
    with BDW[:, r*128:(r+1)*128][4b+i, 4b+j] = blocks[r, b, i, j]."""
    blocks = np.asarray(blocks).astype(np.float32)
    bdw = np.zeros((D, NRELS * D), dtype=np.float32)
    for r in range(NRELS):
        for b in range(NUM_BLOCKS):
            bdw[
                b * BLOCK_SIZE : (b + 1) * BLOCK_SIZE,
                r * D + b * BLOCK_SIZE : r * D + (b + 1) * BLOCK_SIZE,
            ] = blocks[r, b]
    return bdw


def _tiles_per_block(sched):
    return [sum(sg["ntiles"] for sg in blk["sgs"]) for blk in sched]


def _wrap_idxs(src_pad_core, tiles_per_block):
    """Pack per-block gather indices in the dma_gather wrapped layout:
    index j of a block lives at [j % 16, j // 16], replicated across the 8
    groups of 16 partitions. Blocks are concatenated along the free dim.
    Returns [128, Ttot*8] int16."""
    cols = []
    off = 0
    for tb in tiles_per_block:
        ni = int(tb) * TILE_E
        seg = src_pad_core[off : off + ni]
        wrapped = seg.reshape(ni // 16, 16).T  # [16, ni//16]
        cols.append(np.tile(wrapped, (8, 1)))  # [128, ni//16]
        off += ni
    return np.ascontiguousarray(np.concatenate(cols, axis=1))


# ----------------------------------------------------------------------------
# Bass kernel builder (one SPMD program for all cores)

def _build_nc(sched, Ttot):
    tiles_per_block = _tiles_per_block(sched)

    # Bacc (not raw Bass): its compile() pass splits multi-sem waits into
    # EventSemaphores (TRN2 allows 1 wait/instruction), auto-inserts GPSIMD
    # library loads for dma_gather, and encodes extended InstISA subclasses.
    nc = bacc.Bacc("TRN2", target_bir_lowering=False, debug=False, num_devices=N_CORES)

    # fp16 datapath: x table, one-hots, and block-diag weights are fp16
    # (measured matmul rel-err ~3e-4); PSUM accumulation stays fp32.
    # fp16 matmuls run at 1 cycle/row vs 4 for fp32.
    x_d = nc.declare_dram_parameter("x16", [N_NODES, D], F16, isOutput=False)
    srcidx_d = nc.declare_dram_parameter("srcidx", [128, Ttot * 8], I16, isOutput=False)
    # metaf packs [tloc | w] (fp32 tensor_scalar operands) into one DMA;
    # meta16 packs [iota512 | bdw] (fp16). Consumers then depend on few DMAs
    # (ISA sync-wait slots per instruction are scarce).
    metaf_cols = 2 * Ttot
    metaf_d = nc.declare_dram_parameter("metaf", [128, metaf_cols], F32, isOutput=False)
    meta16_cols = 512 + NRELS * D
    meta16_d = nc.declare_dram_parameter("meta16", [128, meta16_cols], F16, isOutput=False)
    out_d = nc.declare_dram_parameter("out", [NBLK * BLK, D], F32, isOutput=True)

    with tile.TileContext(nc) as tc:
        with (
            tc.tile_pool(name="const", bufs=1) as const_pool,
            tc.tile_pool(name="xg", bufs=3) as xg_pool,
            tc.tile_pool(name="oh", bufs=8) as oh_pool,
            tc.tile_pool(name="aggsb", bufs=6) as aggsb_pool,
            tc.tile_pool(name="outsb", bufs=3) as outsb_pool,
            tc.tile_pool(name="psA", bufs=6, space=bass.MemorySpace.PSUM) as psA_pool,
            tc.tile_pool(name="psO", bufs=2, space=bass.MemorySpace.PSUM) as psO_pool,
        ):
            # constants
            srcidx_sb = const_pool.tile([128, Ttot * 8], I16, tag="srcidx")
            nc.sync.dma_start(srcidx_sb[:], srcidx_d[:, :])
            metaf_sb = const_pool.tile([128, metaf_cols], F32, tag="metaf")
            nc.sync.dma_start(metaf_sb[:], metaf_d[:, :])
            meta16_sb = const_pool.tile([128, meta16_cols], F16, tag="meta16")
            nc.sync.dma_start(meta16_sb[:], meta16_d[:, :])
            tloc_sb = metaf_sb[:, 0:Ttot]
            w_sb = metaf_sb[:, Ttot : 2 * Ttot]
            iota_sb = meta16_sb[:, 0:512]
            bdw_sb = meta16_sb[:, 512:]

            tcol = 0       # global tile counter (column into tloc/w)
            scol = 0       # column offset into srcidx (8 cols per tile)
            max_tb = max(tiles_per_block)
            xg_off = 0
            for b in range(NBLK):
                tb = tiles_per_block[b]
                if tb == 0:
                    continue
                ni = tb * TILE_E
                # gather all source rows for this block: [e%128, e//128, din]
                xg = xg_pool.tile([128, max_tb, D], F16, tag="xg")
                nc.gpsimd.dma_gather(
                    out_ap=xg[:, :tb, :],
                    in_ap=x_d[:, :],
                    idxs_ap=srcidx_sb[:, scol : scol + tb * 8],
                    num_idxs=ni,
                    num_idxs_reg=ni,
                    elem_size=D,
                    # single_packet=True caps the index payload at one 2KB
                    # packet (1024 int16 idxs); crashes the device beyond
                    single_packet=False,
                )
                scol += tb * 8
                xg_off = 0

                out_ps = psO_pool.tile([BLK, D], F32, tag="outps")
                n_transforms = sum(len(sg["rels"]) for sg in sched[b]["sgs"])
                gt = xg_off   # tile index within the block gather
                ti = 0        # transform index within block
                # phase 1: all scatter matmuls of the block (keeps every
                # supergroup's PSUM bank live so PE never stalls behind an
                # ACT copy mid-block)
                pending = []
                for sg in sched[b]["sgs"]:
                    mixed = sg["m"] > 0  # merged tiles present
                    agg_ps = psA_pool.tile([D, 4 * BLK], F32, tag="aggps")
                    pending.append((sg, agg_ps))
                    for kind, width, start, stop in sg["tiles"]:
                        if kind == "merged":
                            oh_w = 4 * BLK
                            tgt_ap = agg_ps[:]
                        else:
                            j = int(kind.split("_")[1])
                            oh_w = BLK
                            tgt_ap = agg_ps[:, j * BLK : (j + 1) * BLK]
                        oh = oh_pool.tile([128, 4 * BLK], F16, tag="oh")
                        oh_eng = (
                            nc.gpsimd
                            if (tcol % POOL_OH_EVERY == POOL_OH_EVERY - 1)
                            else nc.vector
                        )
                        oh_eng.tensor_scalar(
                            oh[:, :oh_w],
                            iota_sb[:, :oh_w],
                            tloc_sb[:, tcol : tcol + 1],
                            w_sb[:, tcol : tcol + 1],
                            mybir.AluOpType.is_equal,
                            mybir.AluOpType.mult,
                        )
                        # aggT[din, col] += sum_e xg[e, din] * oh[e, col]
                        nc.tensor.matmul(
                            tgt_ap,
                            xg[:, gt, :],
                            oh[:, :oh_w],
                            start=start,
                            stop=stop,
                            skip_group_check=mixed,
                        )
                        tcol += 1
                        gt += 1
                # phase 2: PSUM->SBUF copies + transform matmuls
                for sg, agg_ps in pending:
                    used = len(sg["rels"]) * BLK
                    agg_sb = aggsb_pool.tile([D, 4 * BLK], F16, tag="aggsb")
                    nc.scalar.copy(agg_sb[:, :used], agg_ps[:, :used])
                    for r in sg["rels"]:
                        j = sg["slots"][r]
                        # out[n, dout] += agg[n, din] @ BDW_r[din, dout]
                        nc.tensor.matmul(
                            out_ps[:],
                            agg_sb[:, j * BLK : (j + 1) * BLK],
                            bdw_sb[:, r * D : (r + 1) * D],
                            start=(ti == 0),
                            stop=(ti == n_transforms - 1),
                        )
                        ti += 1
                xg_off = gt
                out_sb = outsb_pool.tile([BLK, D], F32, tag="outsb")
                nc.scalar.copy(out_sb[:], out_ps[:])
                nc.sync.dma_start(out_d[b * BLK : (b + 1) * BLK, :], out_sb[:])
    nc.compile()
    return nc


# ----------------------------------------------------------------------------

def _make_in_maps(x, sched, Ttot, src_pad, tloc_pad, w_pad, blocks):
    bdw = _make_bdw(blocks)
    iota512 = np.tile(np.arange(512, dtype=np.float32)[None, :], (128, 1))
    tpb = _tiles_per_block(sched)

    x16 = x.astype(np.float16)
    meta16 = np.ascontiguousarray(
        np.concatenate([iota512, bdw], axis=1).astype(np.float16)
    )
    in_maps = []
    for c in range(N_CORES):
        metaf = np.concatenate(
            [tloc_pad[c].reshape(Ttot, 128).T, w_pad[c].reshape(Ttot, 128).T],
            axis=1,
        )
        in_maps.append(
            {
                "x16": x16,
                "srcidx": _wrap_idxs(src_pad[c], tpb),
                "metaf": np.ascontiguousarray(metaf),
                "meta16": meta16,
            }
        )
    return in_maps


def kernel(x, node_keep_mask, source, target, edge_type, edge_weights, blocks):
    global LAST_NC, LAST_IN_MAPS
    x = np.ascontiguousarray(np.asarray(x), dtype=np.float32)
    sched, Ttot, src_pad, tloc_pad, w_pad = _preprocess(
        x, node_keep_mask, source, target, edge_type, edge_weights
    )
    in_maps = _make_in_maps(x, sched, Ttot, src_pad, tloc_pad, w_pad, blocks)
    nc = _build_nc(sched, Ttot)
    LAST_NC, LAST_IN_MAPS = nc, in_maps

    if _DEBUG_SIM:
        from concourse.bass_interp import CoreSim

        outs = []
        for c in range(N_CORES):
            sim = CoreSim(nc)
            for k, v in in_maps[c].items():
                sim.tensor(k)[:] = v
            sim.simulate()
            outs.append(np.array(sim.tensor("out"))[:NPC])
        return np.concatenate(outs, axis=0)

    trace = os.environ.get("KERNEL_TRACE", "0") == "1"
    res = run_bass_kernel_spmd(
        nc, in_maps, core_ids=list(range(N_CORES)), trace=trace
    )
    global LAST_EXEC_TIME_NS
    LAST_EXEC_TIME_NS = res.exec_time_ns
    out = np.concatenate([res.results[c]["out"][:NPC] for c in range(N_CORES)], axis=0)
    return out.astype(np.float32)


LAST_EXEC_TIME_NS = None
LAST_NC = None
LAST_IN_MAPS = None


# revision 52
# speedup vs baseline: 1.0446x; 1.0231x over previous
"""Trainium2 Bass kernel for nn_BlockDecomposition (relational GNN message passing).

Reference computation:
    out[n] = keep[n] * (x[n] @ BD(blocks[-1]))                    (self loop)
           + sum_{directed edge e: tgt_e == n} w_e * (x[src_e] @ BD(blocks[et_e]))
where BD(.) embeds 32 4x4 blocks into a block-diagonal 128x128 matrix and the
edge list is symmetrized (each undirected edge appears in both directions).

Strategy (8 NeuronCores, no collectives):
  - Shard by TARGET node: core c owns nodes [c*1250, (c+1)*1250). Each core
    receives exactly the directed edges targeting its nodes (plus one
    self-loop pseudo-edge per node with relation 16 and weight keep[n]),
    computes its 1250 output rows completely, and the host concatenates.
  - Within a core, nodes are processed in 10 blocks of 128. Per block one
    dma_gather (GPSIMD SWDGE) pulls all needed x rows from the HBM-resident
    fp16 x table into SBUF, laid out [edge mod 128 (partition), tile, 128
    features] -- the gather IS the edge-expansion of x.
  - Relations are organized per block into supergroups of <=4 relation
    "slots" sharing a [din, 4*128] PSUM bank. Each relation contributes
    floor(gmax/128) dense 128-edge "full" tiles; the <=127-edge remainders
    of a supergroup are concatenated into shared 512-wide "merged" tiles
    (one-hot column = 128*slot + tgt_local), eliminating per-relation tail
    padding. Per tile:
      * DVE builds a weighted one-hot OH[e, col] = (iota[col] ==
        tloc[e]) * w[e] in ONE fused tensor_scalar (is_equal, mult), fp16.
      * PE scatter-matmul aggT[din, col] += xg[e, din].T-contract OH[e, col]
        (fp16 x fp16, fp32 PSUM accumulate; 1 cycle/row).
    Per supergroup: one ACT copy moves the PSUM bank to SBUF as fp16; then
    per relation a PE transform matmul out[n, dout] += agg[n, din] @
    BD(W_r)[din, dout] accumulates all 17 relations in a per-block PSUM
    bank, which is copied out (ACT) and DMA'd to the output rows.
  - The schedule (tile counts per cell) is the max over the 8 cores so a
    single SPMD program serves all cores; shorter cores pad with weight-0
    edges. Self-loops ride the same path as relation 16 with w = keep mask.
  - Engine balance (cost model, per core ~53us): DVE ~39us (one-hots),
    Pool ~39us (gather descriptor-gen), PE ~38us (530 matmuls), ACT ~32us
    (PSUM->SBUF copies), all overlapped against ~6.6us/block gather DMA.

Numerics: gathered x, one-hots, and block weights are fp16 (measured HW
matmul rel-err ~3e-4; end-to-end 4.0e-4 vs fp64 reference); accumulation is
fp32 in PSUM. All floating-point arithmetic happens on device. Host work is
index manipulation (sorting/padding/layout), dtype casts, and placing weight
values into the block-diagonal layout.
"""

import os
import sys
import numpy as np

for _p in ("/opt/trn_rl_repo", "/root/.axon_site/_ro/trn_rl_repo"):
    if os.path.isdir(_p) and _p not in sys.path:
        sys.path.insert(0, _p)

import concourse.bass as bass
import concourse.bacc as bacc
import concourse.mybir as mybir
import concourse.tile as tile
from concourse.bass_utils import run_bass_kernel_spmd

# ----------------------------------------------------------------------------
# Problem constants (hardcoded per spec)
N_NODES = 10000
N_EDGES = 160000
NUM_REL = 16          # relations used by edges; blocks[16] is the self-loop
NUM_BLOCKS = 32
BLOCK_SIZE = 4
D = NUM_BLOCKS * BLOCK_SIZE   # 128
N_CORES = 8
NPC = N_NODES // N_CORES      # 1250 nodes per core
BLK = 128                     # node block size (partition dim of scatter)
NBLK = (NPC + BLK - 1) // BLK  # 10 blocks per core (last one partial: 98)
NRELS = NUM_REL + 1           # 16 edge relations + self-loop "relation" 16
TILE_E = 128                  # edges per tile (matmul contraction dim)

F32 = mybir.dt.float32
F16 = mybir.dt.float16
I16 = mybir.dt.int16

# fraction of one-hot builds routed to the GPSIMD (Pool) engine to unload DVE
POOL_OH_EVERY = 1000  # Pool does DMA desc-gen only; all one-hots on DVE

_DEBUG_SIM = os.environ.get("KERNEL_USE_CORESIM", "0") == "1"


# ----------------------------------------------------------------------------
# Host-side preprocessing: integer index manipulation only.

SUPERGROUPS = [list(range(4 * g, 4 * g + 4)) for g in range(4)] + [[NUM_REL]]


def _build_schedule(cnt):
    """Static tile schedule shared by all cores.

    cnt: [C, NBLK, NRELS] per-core (block, rel) edge counts.

    Per block, relations are organized into supergroups of <=4 relation
    "slots" sharing one [din, 512] PSUM bank (slot j at columns 128j). Each
    relation cell contributes floor(gmax/128) dense "full" tiles targeting
    its slot plus a remainder; remainders of a supergroup are concatenated
    and chopped into shared 512-wide "merged" tiles (each edge's one-hot
    column is 128*slot + tloc), which removes per-relation tail padding.

    Returns (sched, Ttot):
      sched: per block dict {
        "sgs": [ { "rels": [r...], "slots": {r: j},
                   "tiles": [ (kind, width, start, stop) ... ]   # in order
                   "cells": {r: (full_tiles, rem)} } ] }
      Ttot: total tile count.
    """
    gmax = cnt.max(axis=0)  # [NBLK, NRELS]
    sched = []
    Ttot = 0
    for b in range(NBLK):
        sgs = []
        for rels_all in SUPERGROUPS:
            rels = [r for r in rels_all if gmax[b, r] > 0]
            if not rels:
                continue
            slots = {r: j for j, r in enumerate(rels)}
            full = {r: int(gmax[b, r]) // TILE_E for r in rels}
            rem = {r: int(gmax[b, r]) % TILE_E for r in rels}
            rem_total = sum(rem.values())
            m = (rem_total + TILE_E - 1) // TILE_E
            n_full = sum(full.values())
            tiles = []
            for i in range(m):
                tiles.append(("merged", TILE_E, i == 0, False))
            for r in rels:
                for t in range(full[r]):
                    # with merged tiles the first merged matmul resets the
                    # whole bank (start), and group bookkeeping is skipped;
                    # without, each slot runs its own start/stop group
                    tiles.append(
                        (
                            "full_%d" % slots[r],
                            TILE_E,
                            m == 0 and t == 0,
                            m == 0 and t == full[r] - 1,
                        )
                    )
            if m > 0:
                tiles[-1] = (tiles[-1][0], tiles[-1][1], tiles[-1][2], True)
            sgs.append(
                {
                    "rels": rels,
                    "slots": slots,
                    "full": full,
                    "rem": rem,
                    "m": m,
                    "ntiles": len(tiles),
                    "tiles": tiles,
                }
            )
            Ttot += len(tiles)
        sched.append({"sgs": sgs})
    return sched, Ttot


def _preprocess(x, node_keep_mask, source, target, edge_type, edge_weights):
    """Build the per-core padded tile schedule.

    Returns:
      sched, Ttot (see _build_schedule), plus per-core arrays:
        src_pad  [C, Ttot*128] int16   source node id per edge slot
        tloc_pad [C, Ttot*128] float32 one-hot column per edge slot
                                        (0..127 full tiles, 0..511 merged)
        w_pad    [C, Ttot*128] float32 edge weight per edge slot (0 for pads)
    """
    src = np.asarray(source).astype(np.int64)
    tgt = np.asarray(target).astype(np.int64)
    et = np.asarray(edge_type).astype(np.int64)
    ew = np.asarray(edge_weights).astype(np.float32)
    keep = np.asarray(node_keep_mask).astype(np.float32)

    # symmetrize + append self-loop pseudo-edges with relation NUM_REL
    nodes = np.arange(N_NODES, dtype=np.int64)
    srcA = np.concatenate([src, tgt, nodes])
    tgtA = np.concatenate([tgt, src, nodes])
    etA = np.concatenate([et, et, np.full(N_NODES, NUM_REL, dtype=np.int64)])
    ewA = np.concatenate([ew, ew, keep])

    core = tgtA // NPC
    loc = tgtA % NPC
    blk = loc // BLK
    tloc = loc % BLK

    # sort by (core, blk, rel); order within a group is irrelevant
    order = np.lexsort((etA, blk, core))
    srcS = srcA[order].astype(np.int16)
    tlocS = tloc[order].astype(np.float32)
    ewS = ewA[order]

    key = (core * NBLK + blk) * NRELS + etA
    cnt = np.bincount(key, minlength=N_CORES * NBLK * NRELS).reshape(
        N_CORES, NBLK, NRELS
    )
    starts = np.concatenate([[0], np.cumsum(cnt.reshape(-1))]).astype(np.int64)

    sched, Ttot = _build_schedule(cnt)

    src_pad = np.zeros((N_CORES, Ttot * TILE_E), dtype=np.int16)
    tloc_pad = np.zeros((N_CORES, Ttot * TILE_E), dtype=np.float32)
    w_pad = np.zeros((N_CORES, Ttot * TILE_E), dtype=np.float32)

    for c in range(N_CORES):
        pos = 0  # edge-slot cursor within this core's stream
        for b in range(NBLK):
            for sg in sched[b]["sgs"]:
                # per-rel edge lists for this core
                seg = {}
                for r in sg["rels"]:
                    gi = (c * NBLK + b) * NRELS + r
                    s0 = int(starts[gi])
                    n = int(cnt[c, b, r])
                    seg[r] = (s0, n)
                # fill order: merged region first (remainder slots of each
                # rel = the edges beyond the full tiles), then full tiles.
                mslots = sg["m"] * TILE_E
                fbase = pos + mslots  # full-tile region start
                # merged region layout: concat over rels of rem[r] slots
                moff = pos
                for r in sg["rels"]:
                    s0, n = seg[r]
                    j = sg["slots"][r]
                    nfull_slots = sg["full"][r] * TILE_E
                    # full tiles take the first min(n, nfull_slots) edges
                    nf = min(n, nfull_slots)
                    src_pad[c, fbase : fbase + nf] = srcS[s0 : s0 + nf]
                    tloc_pad[c, fbase : fbase + nf] = tlocS[s0 : s0 + nf]
                    w_pad[c, fbase : fbase + nf] = ewS[s0 : s0 + nf]
                    fbase += nfull_slots
                    # remainder edges go to this rel's merged slots with
                    # one-hot column 128*j + tloc
                    nr = n - nf
                    assert 0 <= nr <= sg["rem"][r]
                    src_pad[c, moff : moff + nr] = srcS[s0 + nf : s0 + n]
                    tloc_pad[c, moff : moff + nr] = (
                        tlocS[s0 + nf : s0 + n] + 128.0 * j
                    )
                    w_pad[c, moff : moff + nr] = ewS[s0 + nf : s0 + n]
                    moff += sg["rem"][r]
                pos += sg["ntiles"] * TILE_E
        assert pos == Ttot * TILE_E
    return sched, Ttot, src_pad, tloc_pad, w_pad


def _make_bdw(blocks):
    """blocks [17, 32, 4, 4] -> dense block-diagonal lhsT layout [128, 17*128]
    with BDW[:, r*128:(r+1)*128][4b+i, 4b+j] = blocks[r, b, i, j]."""
    blocks = np.asarray(blocks).astype(np.float32)
    bdw = np.zeros((D, NRELS * D), dtype=np.float32)
    for r in range(NRELS):
        for b in range(NUM_BLOCKS):
            bdw[
                b * BLOCK_SIZE : (b + 1) * BLOCK_SIZE,
                r * D + b * BLOCK_SIZE : r * D + (b + 1) * BLOCK_SIZE,
            ] = blocks[r, b]
    return bdw


def _tiles_per_block(sched):
    return [sum(sg["ntiles"] for sg in blk["sgs"]) for blk in sched]


def _wrap_idxs(src_pad_core, tiles_per_block):
    """Pack per-block gather indices in the dma_gather wrapped layout:
    index j of a block lives at [j % 16, j // 16], replicated across the 8
    groups of 16 partitions. Blocks are concatenated along the free dim.
    Returns [128, Ttot*8] int16."""
    cols = []
    off = 0
    for tb in tiles_per_block:
        ni = int(tb) * TILE_E
        seg = src_pad_core[off : off + ni]
        wrapped = seg.reshape(ni // 16, 16).T  # [16, ni//16]
        cols.append(np.tile(wrapped, (8, 1)))  # [128, ni//16]
        off += ni
    return np.ascontiguousarray(np.concatenate(cols, axis=1))


# ----------------------------------------------------------------------------
# Bass kernel builder (one SPMD program for all cores)

def _build_nc(sched, Ttot):
    tiles_per_block = _tiles_per_block(sched)

    # Bacc (not raw Bass): its compile() pass splits multi-sem waits into
    # EventSemaphores (TRN2 allows 1 wait/instruction), auto-inserts GPSIMD
    # library loads for dma_gather, and encodes extended InstISA subclasses.
    nc = bacc.Bacc("TRN2", target_bir_lowering=False, debug=False, num_devices=N_CORES)

    # fp16 datapath: x table, one-hots, and block-diag weights are fp16
    # (measured matmul rel-err ~3e-4); PSUM accumulation stays fp32.
    # fp16 matmuls run at 1 cycle/row vs 4 for fp32.
    x_d = nc.declare_dram_parameter("x16", [N_NODES, D], F16, isOutput=False)
    srcidx_d = nc.declare_dram_parameter("srcidx", [128, Ttot * 8], I16, isOutput=False)
    # metaf packs [tloc | w] (fp32 tensor_scalar operands) into one DMA;
    # meta16 packs [iota512 | bdw] (fp16). Consumers then depend on few DMAs
    # (ISA sync-wait slots per instruction are scarce).
    metaf_cols = 2 * Ttot
    metaf_d = nc.declare_dram_parameter("metaf", [128, metaf_cols], F32, isOutput=False)
    meta16_cols = 512 + NRELS * D
    meta16_d = nc.declare_dram_parameter("meta16", [128, meta16_cols], F16, isOutput=False)
    out_d = nc.declare_dram_parameter("out", [NBLK * BLK, D], F32, isOutput=True)

    with tile.TileContext(nc) as tc:
        with (
            tc.tile_pool(name="const", bufs=1) as const_pool,
            tc.tile_pool(name="xg", bufs=3) as xg_pool,
            tc.tile_pool(name="oh", bufs=2) as oh_pool,
            tc.tile_pool(name="aggsb", bufs=6) as aggsb_pool,
            tc.tile_pool(name="outsb", bufs=3) as outsb_pool,
            tc.tile_pool(name="psA", bufs=5, space=bass.MemorySpace.PSUM) as psA_pool,
            tc.tile_pool(name="psO", bufs=3, space=bass.MemorySpace.PSUM) as psO_pool,
        ):
            # constants
            srcidx_sb = const_pool.tile([128, Ttot * 8], I16, tag="srcidx")
            nc.sync.dma_start(srcidx_sb[:], srcidx_d[:, :])
            metaf_sb = const_pool.tile([128, metaf_cols], F32, tag="metaf")
            nc.sync.dma_start(metaf_sb[:], metaf_d[:, :])
            meta16_sb = const_pool.tile([128, meta16_cols], F16, tag="meta16")
            nc.sync.dma_start(meta16_sb[:], meta16_d[:, :])
            tloc_sb = metaf_sb[:, 0:Ttot]
            w_sb = metaf_sb[:, Ttot : 2 * Ttot]
            iota_sb = meta16_sb[:, 0:512]
            bdw_sb = meta16_sb[:, 512:]

            tcol = 0       # global tile counter (column into tloc/w)
            scol = 0       # column offset into srcidx (8 cols per tile)
            max_tb = max(tiles_per_block)
            xg_off = 0
            for b in range(NBLK):
                tb = tiles_per_block[b]
                if tb == 0:
                    continue
                ni = tb * TILE_E
                # gather all source rows for this block: [e%128, e//128, din]
                xg = xg_pool.tile([128, max_tb, D], F16, tag="xg")
                nc.gpsimd.dma_gather(
                    out_ap=xg[:, :tb, :],
                    in_ap=x_d[:, :],
                    idxs_ap=srcidx_sb[:, scol : scol + tb * 8],
                    num_idxs=ni,
                    num_idxs_reg=ni,
                    elem_size=D,
                    # single_packet=True caps the index payload at one 2KB
                    # packet (1024 int16 idxs); crashes the device beyond
                    single_packet=False,
                )
                scol += tb * 8
                xg_off = 0

                out_ps = psO_pool.tile([BLK, D], F32, tag="outps")
                n_transforms = sum(len(sg["rels"]) for sg in sched[b]["sgs"])
                gt = xg_off   # tile index within the block gather
                ti = 0        # transform index within block
                # one block-sized one-hot arena instead of per-tile tiles:
                # per-tile tiles each cost a DVE EventSemaphore release
                # (~360 of them ~ 10us); one arena costs one
                oh_blk = oh_pool.tile([128, max_tb, 4 * BLK], F16, tag="oh")
                bt = 0       # tile index within this block's oh arena
                # phase 1: all scatter matmuls of the block (keeps every
                # supergroup's PSUM bank live so PE never stalls behind an
                # ACT copy mid-block)
                pending = []
                for sg in sched[b]["sgs"]:
                    mixed = sg["m"] > 0  # merged tiles present
                    agg_ps = psA_pool.tile([D, 4 * BLK], F32, tag="aggps")
                    pending.append((sg, agg_ps))
                    for kind, width, start, stop in sg["tiles"]:
                        if kind == "merged":
                            oh_w = 4 * BLK
                            tgt_ap = agg_ps[:]
                        else:
                            j = int(kind.split("_")[1])
                            oh_w = BLK
                            tgt_ap = agg_ps[:, j * BLK : (j + 1) * BLK]
                        oh = oh_blk[:, bt, :]
                        oh_eng = (
                            nc.gpsimd
                            if (tcol % POOL_OH_EVERY == POOL_OH_EVERY - 1)
                            else nc.vector
                        )
                        oh_eng.tensor_scalar(
                            oh[:, :oh_w],
                            iota_sb[:, :oh_w],
                            tloc_sb[:, tcol : tcol + 1],
                            w_sb[:, tcol : tcol + 1],
                            mybir.AluOpType.is_equal,
                            mybir.AluOpType.mult,
                        )
                        # aggT[din, col] += sum_e xg[e, din] * oh[e, col]
                        nc.tensor.matmul(
                            tgt_ap,
                            xg[:, gt, :],
                            oh[:, :oh_w],
                            start=start,
                            stop=stop,
                            skip_group_check=mixed,
                        )
                        tcol += 1
                        gt += 1
                        bt += 1
                # phase 2: PSUM->SBUF copies + transform matmuls
                for sg, agg_ps in pending:
                    used = len(sg["rels"]) * BLK
                    agg_sb = aggsb_pool.tile([D, 4 * BLK], F16, tag="aggsb")
                    nc.scalar.copy(agg_sb[:, :used], agg_ps[:, :used])
                    for r in sg["rels"]:
                        j = sg["slots"][r]
                        # out[n, dout] += agg[n, din] @ BDW_r[din, dout]
                        nc.tensor.matmul(
                            out_ps[:],
                            agg_sb[:, j * BLK : (j + 1) * BLK],
                            bdw_sb[:, r * D : (r + 1) * D],
                            start=(ti == 0),
                            stop=(ti == n_transforms - 1),
                        )
                        ti += 1
                xg_off = gt
                out_sb = outsb_pool.tile([BLK, D], F32, tag="outsb")
                nc.scalar.copy(out_sb[:], out_ps[:])
                nc.sync.dma_start(out_d[b * BLK : (b + 1) * BLK, :], out_sb[:])
    nc.compile()
    return nc


# ----------------------------------------------------------------------------

def _make_in_maps(x, sched, Ttot, src_pad, tloc_pad, w_pad, blocks):
    bdw = _make_bdw(blocks)
    iota512 = np.tile(np.arange(512, dtype=np.float32)[None, :], (128, 1))
    tpb = _tiles_per_block(sched)

    x16 = x.astype(np.float16)
    meta16 = np.ascontiguousarray(
        np.concatenate([iota512, bdw], axis=1).astype(np.float16)
    )
    in_maps = []
    for c in range(N_CORES):
        metaf = np.concatenate(
            [tloc_pad[c].reshape(Ttot, 128).T, w_pad[c].reshape(Ttot, 128).T],
            axis=1,
        )
        in_maps.append(
            {
                "x16": x16,
                "srcidx": _wrap_idxs(src_pad[c], tpb),
                "metaf": np.ascontiguousarray(metaf),
                "meta16": meta16,
            }
        )
    return in_maps


def kernel(x, node_keep_mask, source, target, edge_type, edge_weights, blocks):
    global LAST_NC, LAST_IN_MAPS
    x = np.ascontiguousarray(np.asarray(x), dtype=np.float32)
    sched, Ttot, src_pad, tloc_pad, w_pad = _preprocess(
        x, node_keep_mask, source, target, edge_type, edge_weights
    )
    in_maps = _make_in_maps(x, sched, Ttot, src_pad, tloc_pad, w_pad, blocks)
    nc = _build_nc(sched, Ttot)
    LAST_NC, LAST_IN_MAPS = nc, in_maps

    if _DEBUG_SIM:
        from concourse.bass_interp import CoreSim

        outs = []
        for c in range(N_CORES):
            sim = CoreSim(nc)
            for k, v in in_maps[c].items():
                sim.tensor(k)[:] = v
            sim.simulate()
            outs.append(np.array(sim.tensor("out"))[:NPC])
        return np.concatenate(outs, axis=0)

    trace = os.environ.get("KERNEL_TRACE", "0") == "1"
    res = run_bass_kernel_spmd(
        nc, in_maps, core_ids=list(range(N_CORES)), trace=trace
    )
    global LAST_EXEC_TIME_NS
    LAST_EXEC_TIME_NS = res.exec_time_ns
    out = np.concatenate([res.results[c]["out"][:NPC] for c in range(N_CORES)], axis=0)
    return out.astype(np.float32)


LAST_EXEC_TIME_NS = None
LAST_NC = None
LAST_IN_MAPS = None
